# revision 1
# baseline (speedup 1.0000x reference)
"""Capsule-routing kernel for Trainium2, data-parallel over batch (8 cores).

Math: the reference's per-instance routing (unique -> gather -> attention)
is reformulated as a dense masked softmax over the 64x64 cell grid:
  - all per-cell quantities (attention keys, value-scalar, activation logit)
    come from one fused per-image GEMM,
  - the relative-position encoding's mean term cancels in the softmax and
    reduces to a rank-1 correction computed from per-instance occupancy sums,
  - per-instance dedup of points is a scatter of ones into a cell bitmap,
  - all 32 instances reduce in a single accumulated PE matmul against the
    occupancy mask.
"""
import sys

sys.path.insert(0, "/opt/trn_rl_repo")

import numpy as np

import concourse.bacc as bacc
import concourse.mybir as mybir
from concourse import masks, tile
from concourse.bass_utils import run_bass_kernel_spmd

F32 = mybir.dt.float32
F32R = mybir.dt.float32r
BF16 = mybir.dt.bfloat16
I32 = mybir.dt.int32
I16 = mybir.dt.int16

B = 8
CIN = 1280
NCELL = 4096  # 64x64 feature grid
NCAPS = 19
NI = 32  # instances per image
NPTS = 256  # points per instance
DK = 64
EPS = 1e-6
NCH = 10  # channel chunks of 128
NJ = 8  # 512-cell column chunks for GEMM1
NCK = 32  # 128-cell chunks

_CACHE = {}

# Force every activation onto the one table set that covers exp/ln/copy so
# the ACT engine never reloads its function tables mid-kernel. Indices of
# act_info.json sets are preserved; other sets are emptied so the inserter
# cannot pick them.
_ONE_SET = "natural_log_exp_and_others"
_orig_get_tables = None


def _patched_tables(arch):
    full = _orig_get_tables(arch)
    return {
        name: (funcs if name == _ONE_SET else set())
        for name, funcs in full.items()
    }


def _install_act_table_patch():
    global _orig_get_tables
    if _orig_get_tables is None:
        _orig_get_tables = bacc.get_activation_tables
        bacc.get_activation_tables = _patched_tables


def _build_nc(dbg=False, loop_n=1, mode="full"):
    key = ("nc", dbg, loop_n, mode)
    if key in _CACHE:
        return _CACHE[key]

    _install_act_table_patch()
    nc = bacc.Bacc(None, target_bir_lowering=False, debug=False)
    if dbg:
        M1D = nc.dram_tensor("M1D", [66, NCELL], F32, kind="ExternalOutput")
        VAD = nc.dram_tensor("VAD", [128, 2 * NCK], F32, kind="ExternalOutput")
        AMLD = nc.dram_tensor("AMLD", [128, NCK], F32, kind="ExternalOutput")
        PS3D = nc.dram_tensor("PS3D", [NI, 40], F32, kind="ExternalOutput")
        OCTD = nc.dram_tensor("OCTD", [128, NCK * NI], F32, kind="ExternalOutput")

    X = nc.dram_tensor("X", [CIN, NCELL], F32, kind="ExternalInput")
    W = nc.dram_tensor("W", [CIN + 3, 66], F32, kind="ExternalInput")
    QT8 = nc.dram_tensor("QT8", [DK, NCAPS], F32, kind="ExternalInput")
    WC2 = nc.dram_tensor("WC2", [128, 2 * NCK], F32, kind="ExternalInput")
    PTS = nc.dram_tensor("PTS", [NI, 2 * NPTS], I32, kind="ExternalInput")
    C3 = nc.dram_tensor("C3", [3, NCELL], F32, kind="ExternalInput")
    OUT = nc.dram_tensor("OUT", [NI, NCAPS], F32, kind="ExternalOutput")

    with tile.TileContext(nc) as tc:
        with (
            tc.tile_pool(name="const", bufs=1) as cpool,
            tc.tile_pool(name="xp", bufs=12) as xpool,
            tc.tile_pool(name="m1", bufs=1) as m1pool,
            tc.tile_pool(name="small", bufs=1) as spool,
            tc.tile_pool(name="ap", bufs=4) as apool,
            tc.tile_pool(name="ps1", bufs=3, space="PSUM") as ps1,
            tc.tile_pool(name="pst", bufs=1, space="PSUM") as pst,
            tc.tile_pool(name="ps2", bufs=2, space="PSUM") as ps2,
            tc.tile_pool(name="pso", bufs=1, space="PSUM") as pso,
            tc.tile_pool(name="ps3", bufs=1, space="PSUM") as ps3,
        ):
            # ---- constants ----
            id128 = cpool.tile([128, 128], F32)
            masks.make_identity(nc, id128[:])
            id16b = cpool.tile([16, 16], BF16)
            masks.make_identity(nc, id16b[:])

            const3 = cpool.tile([3, NCELL], F32R)
            nc.gpsimd.dma_start(const3[:], C3[:].bitcast(F32R))

            # ---- small input DMAs ----
            wsb = cpool.tile([128, 11 * 66], F32R)
            for k in range(NCH):
                nc.gpsimd.dma_start(
                    wsb[:, k * 66 : (k + 1) * 66],
                    W[k * 128 : (k + 1) * 128, :].bitcast(F32R),
                )
            nc.gpsimd.dma_start(
                wsb[0:3, 660:726], W[CIN : CIN + 3, :].bitcast(F32R)
            )
            qsb = cpool.tile([DK, NCAPS], F32)
            nc.gpsimd.dma_start(qsb[:], QT8[:])
            wcsb = cpool.tile([128, 2 * NCK], F32)
            nc.gpsimd.dma_start(wcsb[:], WC2[:])
            # GPSIMD ucode ops must start at partition 0, so the 32
            # instances live as two 16-row groups side by side in the
            # free dimension: layout [16, (group, ...)].
            ptsb = spool.tile([16, 2 * 2 * NPTS], I32)
            nc.gpsimd.dma_start(ptsb[:, 0 : 2 * NPTS], PTS[0:16, :])
            nc.gpsimd.dma_start(ptsb[:, 2 * NPTS : 4 * NPTS], PTS[16:32, :])

            xres = cpool.tile([128, 512], F32R)
            if mode == "compute":
                nc.sync.dma_start(xres[:], X[0:128, 0:512].bitcast(F32R))

            def _dma_body():
                for j in range(NJ):
                    for k in range(NCH):
                        xt = xpool.tile([128, 512], F32R, tag="xt")
                        nc.sync.dma_start(
                            xt[:],
                            X[
                                k * 128 : (k + 1) * 128, j * 512 : (j + 1) * 512
                            ].bitcast(F32R),
                        )

            def body():
                if mode == "dma":
                    _dma_body()
                    return

                # ---- occupancy: keys -> per-quarter int16 idx -> scatter ----
                pv = ptsb[:].rearrange("p (g h f) -> p g h f", g=2, h=2)
                keys = spool.tile([16, 2 * NPTS], I32)
                kx = spool.tile([16, 2 * NPTS], I32)
                kv = keys[:].rearrange("p (g f) -> p g f", g=2)
                kxv = kx[:].rearrange("p (g f) -> p g f", g=2)
                # keys = ((y >> 4) << 6) + (x >> 4)
                nc.vector.tensor_scalar(
                    kv,
                    pv[:, :, 0, :],
                    4,
                    6,
                    op0=mybir.AluOpType.logical_shift_right,
                    op1=mybir.AluOpType.logical_shift_left,
                )
                nc.vector.tensor_scalar(
                    kxv, pv[:, :, 1, :], 4, None,
                    op0=mybir.AluOpType.logical_shift_right,
                )
                nc.vector.tensor_tensor(
                    keys[:], keys[:], kx[:], op=mybir.AluOpType.add
                )

                ones16 = spool.tile([16, NPTS], BF16)
                nc.gpsimd.memset(ones16[:], 1.0)
                occ = spool.tile([16, 2 * NCELL], BF16)

                for q in range(4):
                    t = spool.tile([16, 2 * NPTS], I32, tag="tq")
                    ge = spool.tile([16, 2 * NPTS], I32, tag="geq")
                    lt = spool.tile([16, 2 * NPTS], I32, tag="ltq")
                    nc.vector.tensor_scalar(
                        t[:], keys[:], 1024 * q, None,
                        op0=mybir.AluOpType.subtract,
                    )
                    nc.vector.tensor_scalar(
                        ge[:], t[:], 0, None, op0=mybir.AluOpType.is_ge
                    )
                    nc.vector.tensor_scalar(
                        lt[:], t[:], 1024, None, op0=mybir.AluOpType.is_lt
                    )
                    nc.vector.tensor_tensor(
                        ge[:], ge[:], lt[:], op=mybir.AluOpType.mult
                    )
                    # idx = t + (m * 8192 - 8192): negative outside range
                    nc.vector.tensor_scalar(
                        ge[:], ge[:], 8192, -8192,
                        op0=mybir.AluOpType.mult, op1=mybir.AluOpType.add,
                    )
                    nc.vector.tensor_tensor(
                        t[:], t[:], ge[:], op=mybir.AluOpType.add
                    )
                    idx16 = spool.tile([16, 2 * NPTS], I16, tag="idxq")
                    nc.vector.tensor_copy(idx16[:], t[:])
                    for g in (0, 1):
                        nc.gpsimd.local_scatter(
                            out_ap=occ[
                                :,
                                g * NCELL + q * 1024 : g * NCELL + (q + 1) * 1024,
                            ],
                            data_ap=ones16[:],
                            idxs_ap=idx16[:, g * NPTS : (g + 1) * NPTS],
                            channels=16,
                            num_elems=1024,
                            num_idxs=NPTS,
                        )

                # ---- occ -> occt [128 cells, 32 inst] per chunk ----
                pso_all = pso.tile([128, 2 * NCK * 16], BF16)
                for jj in range(NCK):
                    for g in (0, 1):
                        t16 = (jj * 2 + g) * 16
                        nc.tensor.matmul(
                            pso_all[:, t16 : t16 + 16],
                            occ[
                                :,
                                g * NCELL + jj * 128 : g * NCELL + (jj + 1) * 128,
                            ],
                            id16b[:],
                            is_transpose=True,
                        )
                occt = cpool.tile([128, NCK * NI], F32)
                nc.vector.tensor_copy(occt[:], pso_all[:])

                # ---- main pipeline: per 512-cell column group j ----
                m1 = m1pool.tile([66, NCELL], F32)
                pst_all = pst.tile([128, 2 * NCK], F32)
                va = spool.tile([128, 2 * NCK], F32)
                sg = spool.tile([128, NCK], F32)
                sgw = spool.tile([128, NCK], F32)
                aml = spool.tile([128, NCK], F32)
                psum3 = ps3.tile([NI, 40], F32)
                for jp in range(NJ // 2):
                    psum_a = ps1.tile([66, 512], F32, tag="ps1")
                    psum_b = ps1.tile([66, 512], F32, tag="ps1")
                    psum_pair = [psum_a, psum_b]
                    for k in range(NCH):
                        if mode != "compute":
                            xt = xpool.tile([128, 1024], F32R, tag="xt")
                            nc.sync.dma_start(
                                xt[:],
                                X[
                                    k * 128 : (k + 1) * 128,
                                    jp * 1024 : (jp + 1) * 1024,
                                ].bitcast(F32R),
                            )
                        else:
                            xt = xres
                        for h in (0, 1):
                            nc.tensor.matmul(
                                psum_pair[h][:],
                                wsb[:, k * 66 : (k + 1) * 66],
                                xt[:, h * 512 : (h + 1) * 512],
                                start=(k == 0),
                                stop=False,
                            )
                    for h in (0, 1):
                        nc.tensor.matmul(
                            psum_pair[h][:],
                            wsb[0:3, 660:726],
                            const3[:, (2 * jp + h) * 512 : (2 * jp + h + 1) * 512],
                            start=False,
                            stop=True,
                        )
                    for h in (0, 1):
                        j = 2 * jp + h
                        nc.scalar.copy(
                            m1[:, j * 512 : (j + 1) * 512], psum_pair[h][:]
                        )
                        # transpose [vl; alogit] for this group's 4 chunks
                        for s in range(4):
                            jj = 4 * j + s
                            cs = slice(jj * 128, (jj + 1) * 128)
                            nc.tensor.matmul(
                                pst_all[:, 2 * jj : 2 * jj + 2],
                                m1[64:66, cs],
                                id128[64:66, 64:66],
                                is_transpose=True,
                            )
                        nc.vector.tensor_copy(
                            va[:, 8 * j : 8 * j + 8], pst_all[:, 8 * j : 8 * j + 8]
                        )
                        vav = va[:].rearrange("p (c two) -> p c two", two=2)
                        js = slice(4 * j, 4 * j + 4)
                        # am_l = ln(sigmoid(z)+eps) = ln(1+eps+eps*e^-z) - ln(1+e^-z)
                        # using only exp/ln so ACT stays on one function-table set
                        nc.scalar.activation(
                            sg[:, js], vav[:, js, 1],
                            mybir.ActivationFunctionType.Exp, scale=-1.0,
                        )
                        nc.vector.tensor_scalar(
                            sg[:, js], sg[:, js], 1.0, None, op0=mybir.AluOpType.add
                        )
                        nc.vector.tensor_scalar(
                            sgw[:, js], sg[:, js], EPS, 1.0,
                            op0=mybir.AluOpType.mult, op1=mybir.AluOpType.add,
                        )
                        nc.scalar.activation(
                            sg[:, js], sg[:, js], mybir.ActivationFunctionType.Ln
                        )
                        nc.scalar.activation(
                            sgw[:, js], sgw[:, js], mybir.ActivationFunctionType.Ln
                        )
                        nc.vector.tensor_tensor(
                            aml[:, js], sgw[:, js], sg[:, js],
                            op=mybir.AluOpType.subtract,
                        )

                        # scores + A-tiles for the 4 chunks
                        ats = []
                        for s in range(4):
                            jj = 4 * j + s
                            cs = slice(jj * 128, (jj + 1) * 128)
                            psum2 = ps2.tile([128, NCAPS], F32, tag="ps2")
                            nc.tensor.matmul(psum2[:], m1[0:64, cs], qsb[:])
                            at = apool.tile([128, 40], F32, tag="at")
                            nc.scalar.activation(
                                at[:, 0:NCAPS],
                                psum2[:],
                                mybir.ActivationFunctionType.Exp,
                                bias=aml[:, jj : jj + 1],
                            )
                            nc.vector.tensor_scalar(
                                at[:, NCAPS : 2 * NCAPS],
                                at[:, 0:NCAPS],
                                va[:, 2 * jj : 2 * jj + 1],
                                None,
                                op0=mybir.AluOpType.mult,
                            )
                            nc.vector.tensor_copy(
                                at[:, 38:40], wcsb[:, 2 * jj : 2 * jj + 2]
                            )
                            ats.append(at)
                        for s in range(4):
                            jj = 4 * j + s
                            nc.tensor.matmul(
                                psum3[:],
                                occt[:, jj * NI : (jj + 1) * NI],
                                ats[s][:],
                                start=(jj == 0),
                                stop=(jj == NCK - 1),
                            )


                # ---- finalize: sigmoid(num/den + corr/n) ----
                rsb = spool.tile([NI, 40], F32)
                nc.scalar.copy(rsb[:], psum3[:])
                if dbg:
                    nc.sync.dma_start(M1D[:], m1[:])
                    nc.sync.dma_start(VAD[:], va[:])
                    nc.sync.dma_start(AMLD[:], aml[:])
                    nc.sync.dma_start(PS3D[:], rsb[:])
                    nc.sync.dma_start(OCTD[:], occt[:])
                t1 = spool.tile([NI, NCAPS], F32)
                t2 = spool.tile([NI, 1], F32)
                rc1 = spool.tile([NI, NCAPS], F32)
                rc2 = spool.tile([NI, 1], F32)
                nc.vector.reciprocal(rc1[:], rsb[:, 0:NCAPS])
                nc.vector.tensor_tensor(
                    t1[:], rsb[:, NCAPS : 2 * NCAPS], rc1[:],
                    op=mybir.AluOpType.mult,
                )
                nc.vector.reciprocal(rc2[:], rsb[:, 39:40])
                nc.vector.tensor_tensor(
                    t2[:], rsb[:, 38:39], rc2[:], op=mybir.AluOpType.mult
                )
                nc.vector.tensor_scalar(
                    t1[:], t1[:], t2[:], None, op0=mybir.AluOpType.add
                )
                # sigmoid(L) = exp(-ln(1+exp(-L))) with only exp/ln
                osb = spool.tile([NI, NCAPS], F32)
                nc.scalar.activation(
                    osb[:], t1[:], mybir.ActivationFunctionType.Exp, scale=-1.0
                )
                nc.vector.tensor_scalar(
                    osb[:], osb[:], 1.0, None, op0=mybir.AluOpType.add
                )
                nc.scalar.activation(
                    osb[:], osb[:], mybir.ActivationFunctionType.Ln
                )
                nc.scalar.activation(
                    osb[:], osb[:], mybir.ActivationFunctionType.Exp, scale=-1.0
                )
                nc.sync.dma_start(OUT[:], osb[:])

            if loop_n == 1:
                body()
            else:
                with tc.For_i(0, loop_n, 1):
                    body()

    nc.compile()
    _CACHE[key] = nc
    return nc


def _fold_weights(Wp, bp, Wa, ba, Q, Wk, bk, Wv, bv, Wl, bl):
    f = lambda t: np.asarray(t, np.float64)
    Wp, bp, Wa, ba, Q, Wk, bk, Wv, bv, Wl, bl = map(
        f, (Wp, bp, Wa, ba, Q, Wk, bk, Wv, bv, Wl, bl)
    )
    wl = Wl[:, 0]
    WK = Wp.T @ Wk[:256]
    wvl_cap = Wv[:256] @ wl
    a, b = Wv[256] @ wl, Wv[257] @ wl

    W_all = np.zeros((CIN + 3, 66), np.float64)
    W_all[:CIN, :64] = WK
    W_all[:CIN, 64] = Wp.T @ wvl_cap
    W_all[:CIN, 65] = Wa[0]
    W_all[CIN + 0, :64] = Wk[256] / 64.0
    W_all[CIN + 1, :64] = Wk[257] / 64.0
    W_all[CIN + 2, :64] = bp @ Wk[:256] + bk
    W_all[CIN + 0, 64] = a / 64.0
    W_all[CIN + 1, 64] = b / 64.0
    W_all[CIN + 2, 64] = bp @ wvl_cap + bv @ wl
    W_all[CIN + 2, 65] = ba[0]

    c = np.arange(NCELL)
    y64 = (c // 64) / 64.0
    x64 = (c % 64) / 64.0
    wcorr = -(a * y64 + b * x64 - bl[0])
    WC2 = np.empty((128, 2 * NCK), np.float64)
    WC2[:, 0::2] = wcorr.reshape(NCK, 128).T
    WC2[:, 1::2] = 1.0

    return (
        W_all.astype(np.float32),
        (Q.T / 8.0).astype(np.float32),
        WC2.astype(np.float32),
    )


def _make_in_maps(
    feature_output, Wp, bp, Wa, ba, Q, Wk, bk, Wv, bv, Wl, bl, point_lists
):
    W_all, QT8, WC2 = _fold_weights(Wp, bp, Wa, ba, Q, Wk, bk, Wv, bv, Wl, bl)

    c = np.arange(NCELL)
    C3v = np.stack([c // 64, c % 64, np.ones(NCELL)]).astype(np.float32)

    fo = np.ascontiguousarray(np.asarray(feature_output, np.float32))
    pts = np.ascontiguousarray(np.asarray(point_lists).astype(np.int32))

    return [
        {
            "X": fo[i].reshape(CIN, NCELL),
            "W": W_all,
            "QT8": QT8,
            "WC2": WC2,
            "PTS": pts[i].reshape(NI, 2 * NPTS),
            "C3": C3v,
        }
        for i in range(B)
    ]


def kernel(
    feature_output, Wp, bp, Wa, ba, Q, Wk, bk, Wv, bv, Wl, bl, point_lists
):
    nc = _build_nc()
    in_maps = _make_in_maps(
        feature_output, Wp, bp, Wa, ba, Q, Wk, bk, Wv, bv, Wl, bl, point_lists
    )
    res = run_bass_kernel_spmd(nc, in_maps, core_ids=list(range(B)))
    return np.stack([res.results[i]["OUT"] for i in range(B)]).astype(np.float32)



# revision 11
# speedup vs baseline: 1.7533x; 1.7533x over previous
"""Capsule-routing kernel for Trainium2, data-parallel over batch (8 cores).

Math: the reference's per-instance routing (unique -> gather -> attention)
is reformulated as a dense masked softmax over the 64x64 cell grid:
  - Q is folded into the 1x1-conv weights host-side, so one fused per-image
    GEMM (21 output channels: 19 score caps + value-scalar + act-logit)
    produces raw attention scores directly,
  - the relative-position encoding's mean term cancels in the softmax and
    reduces to a rank-1 correction computed from per-instance occupancy sums,
  - per-instance dedup of points is a scatter of ones into a cell bitmap
    (128 gpsimd channels, 4 point-groups per instance, merged by a 4->1
    reduction matmul and clamped to 1),
  - all 32 instances reduce in a single accumulated PE matmul against the
    occupancy mask.
X is cast to fp16 host-side (halves HBM traffic and PE stream cycles);
attention weights run in bf16; accumulation stays fp32 in PSUM.
"""
import sys

sys.path.insert(0, "/opt/trn_rl_repo")

import numpy as np

import concourse.bacc as bacc
import concourse.mybir as mybir
from concourse import masks, tile
from concourse.bass_utils import run_bass_kernel_spmd

F32 = mybir.dt.float32
F16 = mybir.dt.float16
BF16 = mybir.dt.bfloat16
I32 = mybir.dt.int32
I16 = mybir.dt.int16

B = 8
CIN = 1280
NCELL = 4096  # 64x64 feature grid
NCAPS = 19
NI = 32  # instances per image
NPTS = 256  # points per instance
EPS = 1e-6
NCH = 10  # channel chunks of 128
NJ = 8  # 512-cell column groups
NCK = 32  # 128-cell chunks
NO = 21  # fused GEMM outputs: 19 scores + vl + alogit
NOP = 24  # padded output width (keeps DMA slices 16B-aligned)

_CACHE = {}

# Force every activation onto the one table set that covers exp/ln/copy so
# the ACT engine never reloads its function tables mid-kernel.
_ONE_SET = "natural_log_exp_and_others"
_orig_get_tables = None


def _patched_tables(arch):
    full = _orig_get_tables(arch)
    return {
        name: (funcs if name == _ONE_SET else set())
        for name, funcs in full.items()
    }


def _install_act_table_patch():
    global _orig_get_tables
    if _orig_get_tables is None:
        _orig_get_tables = bacc.get_activation_tables
        bacc.get_activation_tables = _patched_tables


def _build_nc(dbg=False, loop_n=1, mode="full"):
    key = ("nc", dbg, loop_n, mode)
    if key in _CACHE:
        return _CACHE[key]

    _install_act_table_patch()
    nc = bacc.Bacc(None, target_bir_lowering=False, debug=False)
    if dbg:
        M1D = nc.dram_tensor("M1D", [NO, NCELL], F32, kind="ExternalOutput")
        OCTD = nc.dram_tensor("OCTD", [128, NCK * NI], F32, kind="ExternalOutput")
        PS3D = nc.dram_tensor("PS3D", [NI, 40], F32, kind="ExternalOutput")

    X = nc.dram_tensor("X", [CIN, NCELL], F16, kind="ExternalInput")
    W = nc.dram_tensor("W", [CIN + 3, NOP], F16, kind="ExternalInput")
    WC2 = nc.dram_tensor("WC2", [128, 2 * NCK], F32, kind="ExternalInput")
    # host-deduped scatter indices: [4*inst+grp, 4 quarters * 64 idxs] int16,
    # value = cell - 1024*q within its quarter, negative = skip.
    PTSI = nc.dram_tensor("PTSI", [128, 256], I16, kind="ExternalInput")
    S4D = nc.dram_tensor("S4D", [128, NI], BF16, kind="ExternalInput")
    C3 = nc.dram_tensor("C3", [3, NCELL], F16, kind="ExternalInput")
    OUT = nc.dram_tensor("OUT", [NI, NCAPS], F32, kind="ExternalOutput")

    with tile.TileContext(nc) as tc:
        with (
            tc.tile_pool(name="const", bufs=1) as cpool,
            tc.tile_pool(name="xp", bufs=40) as xpool,
            tc.tile_pool(name="m1", bufs=1) as m1pool,
            tc.tile_pool(name="small", bufs=1) as spool,
            tc.tile_pool(name="grp", bufs=2) as gpool,
            tc.tile_pool(name="ps1", bufs=3, space="PSUM") as ps1,
            tc.tile_pool(name="pst", bufs=2, space="PSUM") as pstp,
            tc.tile_pool(name="pso", bufs=1, space="PSUM") as pso,
            tc.tile_pool(name="ps3", bufs=1, space="PSUM") as ps3,
        ):
            # ---- constants / small DMAs (rings; gpsimd stays free) ----
            ptsi = spool.tile([128, 256], I16)
            nc.sync.dma_start(ptsi[:], PTSI[:])

            id128 = cpool.tile([128, 128], F32)
            masks.make_identity(nc, id128[:])

            wsb = cpool.tile([128, 11 * NOP], F16)
            for k in range(NCH):
                nc.sync.dma_start(
                    wsb[:, k * NOP : (k + 1) * NOP],
                    W[k * 128 : (k + 1) * 128, :],
                )
            nc.sync.dma_start(
                wsb[0:3, 10 * NOP : 10 * NOP + NOP], W[CIN : CIN + 3, :]
            )
            const3 = cpool.tile([3, NCELL], F16)
            nc.sync.dma_start(const3[:], C3[:])
            wcsb = cpool.tile([128, 2 * NCK], F32)
            nc.sync.dma_start(wcsb[:], WC2[:])
            s4sb = cpool.tile([128, NI], BF16)
            nc.sync.dma_start(s4sb[:], S4D[:])

            xres = cpool.tile([128, 1024], F16)
            if mode == "compute":
                nc.sync.dma_start(xres[:], X[0:128, 0:1024])

            def _x_dma(jp, k):
                xt = xpool.tile([128, 1024], F16, tag="xt")
                nc.sync.dma_start(
                    xt[:],
                    X[k * 128 : (k + 1) * 128, jp * 1024 : (jp + 1) * 1024],
                )
                return xt

            def body():
                if mode == "dma":
                    for jp in range(NJ // 2):
                        for k in range(NCH):
                            _x_dma(jp, k)
                    return

                # issue every X tile DMA up front: 40 resident tiles,
                # zero WAR recycling, the rings just stream.
                xts = {}
                if mode != "compute":
                    for jp in range(NJ // 2):
                        for k in range(NCH):
                            xts[(jp, k)] = _x_dma(jp, k)

                # ---- occupancy: scatter host-deduped indices ----
                ones128 = spool.tile([128, 64], BF16)
                nc.gpsimd.memset(ones128[:], 1.0)
                occ = spool.tile([128, NCELL], BF16)
                for q in range(4):
                    nc.gpsimd.local_scatter(
                        out_ap=occ[:, q * 1024 : (q + 1) * 1024],
                        data_ap=ones128[:],
                        idxs_ap=ptsi[:, q * 64 : (q + 1) * 64],
                        channels=128,
                        num_elems=1024,
                        num_idxs=64,
                    )

                # occ [4*inst+grp, cell] -> occt [cell, inst] (4->1 merge via
                # S4 then clamp to 1). Emitted interleaved with the main loop
                # (quarter q right before group 2q) to keep PE dense.
                pso_all = pso.tile([128, NCK * NI], F32)
                occt = cpool.tile([128, NCK * NI], BF16)

                def occ_quarter(q):
                    for s in range(8):
                        jj = 8 * q + s
                        nc.tensor.matmul(
                            pso_all[:, jj * NI : (jj + 1) * NI],
                            occ[:, jj * 128 : (jj + 1) * 128],
                            s4sb[:],
                        )
                    # clamp merged counts to 1 while copying PSUM->SBUF
                    nc.vector.tensor_scalar(
                        occt[:, q * 8 * NI : (q + 1) * 8 * NI],
                        pso_all[:, q * 8 * NI : (q + 1) * 8 * NI],
                        1.0,
                        None,
                        op0=mybir.AluOpType.min,
                    )

                # at_all: per chunk jj cols [40jj,40jj+40):
                #   0:19 exp(score+aml), 19:38 *vl, 38:40 (wcorr, 1)
                at_all = cpool.tile([128, NCK * 40], BF16)
                atv = at_all[:].rearrange("p (c k) -> p c k", k=40)
                nc.vector.tensor_copy(atv[:, :, 38:40], wcsb[:])

                m1 = m1pool.tile([NOP, NCELL], F32)
                psum3 = ps3.tile([NI, 40], F32)
                for jp in range(NJ // 2):
                    if jp < 2:
                        occ_quarter(jp)
                    psum_a = ps1.tile([NOP, 512], F32, tag="ps1")
                    psum_b = ps1.tile([NOP, 512], F32, tag="ps1")
                    psum_pair = [psum_a, psum_b]
                    for k in range(NCH):
                        xt = xts[(jp, k)] if mode != "compute" else xres
                        for h in (0, 1):
                            nc.tensor.matmul(
                                psum_pair[h][:],
                                wsb[:, k * NOP : (k + 1) * NOP],
                                xt[:, h * 512 : (h + 1) * 512],
                                start=(k == 0),
                                stop=False,
                            )
                    for h in (0, 1):
                        nc.tensor.matmul(
                            psum_pair[h][:],
                            wsb[0:3, 10 * NOP : 10 * NOP + NOP],
                            const3[:, (2 * jp + h) * 512 : (2 * jp + h + 1) * 512],
                            start=False,
                            stop=True,
                        )
                    if jp >= 2:
                        occ_quarter(jp)
                    for h in (0, 1):
                        j = 2 * jp + h
                        nc.vector.tensor_copy(
                            m1[:, j * 512 : (j + 1) * 512], psum_pair[h][:]
                        )
                        # transpose the 4 chunks of this group: [21,128]->[128,21]
                        pst = pstp.tile([128, 4 * NOP], F32, tag="pst")
                        for s in range(4):
                            jj = 4 * j + s
                            nc.tensor.matmul(
                                pst[:, s * NOP : (s + 1) * NOP],
                                m1[:, jj * 128 : (jj + 1) * 128],
                                id128[0:NOP, 0:NOP],
                                is_transpose=True,
                            )
                        # gather vl / alogit columns, batch aml for 4 chunks
                        pv = pst[:].rearrange("p (s k) -> p s k", k=NOP)
                        vls = gpool.tile([128, 4], F32, tag="vls")
                        av = gpool.tile([128, 4], F32, tag="av")
                        nc.vector.tensor_copy(vls[:], pv[:, :, 19])
                        nc.vector.tensor_copy(av[:], pv[:, :, 20])
                        # aml = ln(sigmoid(z)+eps) = ln(1+eps+eps*e^-z)-ln(1+e^-z)
                        sg = gpool.tile([128, 4], F32, tag="sg")
                        sgw = gpool.tile([128, 4], F32, tag="sgw")
                        nc.scalar.activation(
                            sg[:], av[:],
                            mybir.ActivationFunctionType.Exp, scale=-1.0,
                        )
                        nc.vector.tensor_scalar(
                            sg[:], sg[:], 1.0, None, op0=mybir.AluOpType.add
                        )
                        nc.vector.tensor_scalar(
                            sgw[:], sg[:], EPS, 1.0,
                            op0=mybir.AluOpType.mult, op1=mybir.AluOpType.add,
                        )
                        nc.scalar.activation(
                            sg[:], sg[:], mybir.ActivationFunctionType.Ln
                        )
                        nc.scalar.activation(
                            sgw[:], sgw[:], mybir.ActivationFunctionType.Ln
                        )
                        aml = gpool.tile([128, 4], F32, tag="aml")
                        nc.vector.tensor_tensor(
                            aml[:], sgw[:], sg[:], op=mybir.AluOpType.subtract
                        )
                        for s in range(4):
                            jj = 4 * j + s
                            nc.scalar.activation(
                                atv[:, jj, 0:19],
                                pst[:, s * NOP : s * NOP + 19],
                                mybir.ActivationFunctionType.Exp,
                                bias=aml[:, s : s + 1],
                            )
                            nc.vector.tensor_scalar(
                                atv[:, jj, 19:38],
                                atv[:, jj, 0:19],
                                vls[:, s : s + 1],
                                None,
                                op0=mybir.AluOpType.mult,
                            )
                        for s in range(4):
                            jj = 4 * j + s
                            nc.tensor.matmul(
                                psum3[:],
                                occt[:, jj * NI : (jj + 1) * NI],
                                atv[:, jj, :],
                                start=(jj == 0),
                                stop=(jj == NCK - 1),
                            )

                # ---- finalize: sigmoid(num/den + corr/n) ----
                rsb = spool.tile([NI, 40], F32)
                nc.scalar.copy(rsb[:], psum3[:])
                if dbg:
                    nc.sync.dma_start(M1D[:], m1[:])
                    nc.sync.dma_start(OCTD[:], occt[:])
                    nc.sync.dma_start(PS3D[:], rsb[:])
                t1 = spool.tile([NI, NCAPS], F32)
                t2 = spool.tile([NI, 1], F32)
                rc1 = spool.tile([NI, NCAPS], F32)
                rc2 = spool.tile([NI, 1], F32)
                nc.vector.reciprocal(rc1[:], rsb[:, 0:NCAPS])
                nc.vector.tensor_tensor(
                    t1[:], rsb[:, NCAPS : 2 * NCAPS], rc1[:],
                    op=mybir.AluOpType.mult,
                )
                nc.vector.reciprocal(rc2[:], rsb[:, 39:40])
                nc.vector.tensor_tensor(
                    t2[:], rsb[:, 38:39], rc2[:], op=mybir.AluOpType.mult
                )
                nc.vector.tensor_scalar(
                    t1[:], t1[:], t2[:], None, op0=mybir.AluOpType.add
                )
                # sigmoid(L) = exp(-ln(1+exp(-L))) with only exp/ln
                osb = spool.tile([NI, NCAPS], F32)
                nc.scalar.activation(
                    osb[:], t1[:], mybir.ActivationFunctionType.Exp, scale=-1.0
                )
                nc.vector.tensor_scalar(
                    osb[:], osb[:], 1.0, None, op0=mybir.AluOpType.add
                )
                nc.scalar.activation(
                    osb[:], osb[:], mybir.ActivationFunctionType.Ln
                )
                nc.scalar.activation(
                    osb[:], osb[:], mybir.ActivationFunctionType.Exp, scale=-1.0
                )
                nc.sync.dma_start(OUT[:], osb[:])

            if loop_n == 1:
                body()
            else:
                with tc.For_i(0, loop_n, 1):
                    body()

    nc.compile()
    _CACHE[key] = nc
    return nc


def _fold_weights(Wp, bp, Wa, ba, Q, Wk, bk, Wv, bv, Wl, bl):
    f = lambda t: np.asarray(t, np.float64)
    Wp, bp, Wa, ba, Q, Wk, bk, Wv, bv, Wl, bl = map(
        f, (Wp, bp, Wa, ba, Q, Wk, bk, Wv, bv, Wl, bl)
    )
    wl = Wl[:, 0]
    QT8 = Q.T / 8.0                       # [64,19]
    WK = Wp.T @ Wk[:256]                  # [1280,64]
    wvl_cap = Wv[:256] @ wl               # [256]
    a, b = Wv[256] @ wl, Wv[257] @ wl

    W_all = np.zeros((CIN + 3, NOP), np.float64)
    W_all[:CIN, 0:19] = WK @ QT8
    W_all[:CIN, 19] = Wp.T @ wvl_cap
    W_all[:CIN, 20] = Wa[0]
    W_all[CIN + 0, 0:19] = (Wk[256] / 64.0) @ QT8
    W_all[CIN + 1, 0:19] = (Wk[257] / 64.0) @ QT8
    W_all[CIN + 2, 0:19] = (bp @ Wk[:256] + bk) @ QT8
    W_all[CIN + 0, 19] = a / 64.0
    W_all[CIN + 1, 19] = b / 64.0
    W_all[CIN + 2, 19] = bp @ wvl_cap + bv @ wl
    W_all[CIN + 2, 20] = ba[0]

    c = np.arange(NCELL)
    y64 = (c // 64) / 64.0
    x64 = (c % 64) / 64.0
    wcorr = -(a * y64 + b * x64 - bl[0])
    WC2 = np.empty((128, 2 * NCK), np.float64)
    WC2[:, 0::2] = wcorr.reshape(NCK, 128).T
    WC2[:, 1::2] = 1.0

    return W_all.astype(np.float16), WC2.astype(np.float32)


def _make_in_maps(
    feature_output, Wp, bp, Wa, ba, Q, Wk, bk, Wv, bv, Wl, bl, point_lists
):
    import ml_dtypes

    W_all, WC2 = _fold_weights(Wp, bp, Wa, ba, Q, Wk, bk, Wv, bv, Wl, bl)

    c = np.arange(NCELL)
    C3v = np.stack([c // 64, c % 64, np.ones(NCELL)]).astype(np.float16)

    S4 = np.zeros((128, NI), np.float32)
    S4[np.arange(128), np.arange(128) // 4] = 1.0
    S4 = S4.astype(ml_dtypes.bfloat16)

    fo = np.asarray(feature_output, np.float32).astype(np.float16)

    # Host-deduped scatter indices. For each instance: unique cell keys,
    # split across its 4 gpsimd channels (partition 4*i+g), then per
    # quarter q the value is cell-1024q if the cell lies in that quarter
    # else -1 (negative indices are skipped by local_scatter).
    pl = np.asarray(point_lists).astype(np.int64)  # [B, NI, 2, 256]
    keys = (pl[:, :, 0] // 16) * 64 + (pl[:, :, 1] // 16)  # [B, NI, 256]
    ptsi = np.full((B, 128, 256), -1, np.int16)
    for i in range(B):
        for n in range(NI):
            u = np.unique(keys[i, n])
            for g in range(4):
                seg = u[64 * g : 64 * (g + 1)]
                if seg.size == 0:
                    continue
                q = seg // 1024
                ptsi[i, 4 * n + g, q * 64 + np.arange(seg.size) % 64] = (
                    seg - 1024 * q
                )
    return [
        {
            "X": np.ascontiguousarray(fo[i].reshape(CIN, NCELL)),
            "W": W_all,
            "WC2": WC2,
            "PTSI": ptsi[i],
            "S4D": S4,
            "C3": C3v,
        }
        for i in range(B)
    ]


def kernel(
    feature_output, Wp, bp, Wa, ba, Q, Wk, bk, Wv, bv, Wl, bl, point_lists
):
    nc = _build_nc()
    in_maps = _make_in_maps(
        feature_output, Wp, bp, Wa, ba, Q, Wk, bk, Wv, bv, Wl, bl, point_lists
    )
    res = run_bass_kernel_spmd(nc, in_maps, core_ids=list(range(B)))
    return np.stack([res.results[i]["OUT"] for i in range(B)]).astype(np.float32)


# revision 12
# speedup vs baseline: 2.0997x; 1.1975x over previous
"""Capsule-routing kernel for Trainium2, data-parallel over batch (8 cores).

Math: the reference's per-instance routing (unique -> gather -> attention)
is reformulated as a dense masked softmax over the 64x64 cell grid:
  - Q is folded into the 1x1-conv weights host-side, so one fused per-image
    GEMM (21 output channels: 19 score caps + value-scalar + act-logit)
    produces raw attention scores directly,
  - the relative-position encoding's mean term cancels in the softmax and
    reduces to a rank-1 correction computed from per-instance occupancy sums,
  - per-instance dedup of points happens host-side; the device scatters the
    unique cell ids of each instance (4 gpsimd channels per instance) into a
    bitmap, merged by a 4->1 reduction matmul and clamped to 1,
  - all 32 instances reduce in a single accumulated PE matmul against the
    occupancy mask.
X is cast to fp16 host-side (halves HBM traffic and PE stream cycles);
attention weights run in bf16; accumulation stays fp32 in PSUM.
DMA discipline: dma_start is a blocking ~0.6us instruction on the issuing
sequencer, so X moves as 20 big [128,2048] tiles split across the sync and
gpsimd sequencers, and all small tensors ride one host-packed byte blob.
"""
import sys

sys.path.insert(0, "/opt/trn_rl_repo")

import numpy as np

import concourse.bacc as bacc
import concourse.mybir as mybir
from concourse import masks, tile
from concourse.bass_utils import run_bass_kernel_spmd

F32 = mybir.dt.float32
F16 = mybir.dt.float16
BF16 = mybir.dt.bfloat16
I16 = mybir.dt.int16
U8 = mybir.dt.uint8

B = 8
CIN = 1280
NCELL = 4096  # 64x64 feature grid
NCAPS = 19
NI = 32  # instances per image
EPS = 1e-6
NCH = 10  # channel chunks of 128
NCK = 32  # 128-cell chunks
NO = 21  # fused GEMM outputs: 19 scores + vl + alogit
NOP = 24  # padded output width (keeps slices 16B-aligned)

# byte offsets in the packed small-tensor blob [128, PACKB] u8
OFF_W = 0          # fp16 [128, 264]: 10 x [128,24] chunks + [3,24] tail chunk
OFF_WC = 528       # f32  [128, 64]: interleaved (wcorr, 1) per cell chunk
OFF_S4 = 784       # bf16 [128, 32]: 4->1 group merge matrix
OFF_PT = 848       # i16  [128, 256]: deduped scatter idxs, 4 quarters x 64
PACKB = 1360

_CACHE = {}

# Force every activation onto the one table set that covers exp/ln/copy so
# the ACT engine never reloads its function tables mid-kernel.
_ONE_SET = "natural_log_exp_and_others"
_orig_get_tables = None


def _patched_tables(arch):
    full = _orig_get_tables(arch)
    return {
        name: (funcs if name == _ONE_SET else set())
        for name, funcs in full.items()
    }


def _install_act_table_patch():
    global _orig_get_tables
    if _orig_get_tables is None:
        _orig_get_tables = bacc.get_activation_tables
        bacc.get_activation_tables = _patched_tables


def _build_nc(dbg=False, loop_n=1, mode="full"):
    key = ("nc", dbg, loop_n, mode)
    if key in _CACHE:
        return _CACHE[key]

    _install_act_table_patch()
    nc = bacc.Bacc(None, target_bir_lowering=False, debug=False)
    if dbg:
        M1D = nc.dram_tensor("M1D", [NOP, NCELL], F32, kind="ExternalOutput")
        OCTD = nc.dram_tensor("OCTD", [128, NCK * NI], F32, kind="ExternalOutput")
        PS3D = nc.dram_tensor("PS3D", [NI, 40], F32, kind="ExternalOutput")

    X = nc.dram_tensor("X", [CIN, NCELL], F16, kind="ExternalInput")
    PACK = nc.dram_tensor("PACK", [128, PACKB], U8, kind="ExternalInput")
    C3 = nc.dram_tensor("C3", [3, NCELL], F16, kind="ExternalInput")
    OUT = nc.dram_tensor("OUT", [NI, NCAPS], F32, kind="ExternalOutput")

    with tile.TileContext(nc) as tc:
        with (
            tc.tile_pool(name="const", bufs=1) as cpool,
            tc.tile_pool(name="xp", bufs=20) as xpool,
            tc.tile_pool(name="m1", bufs=1) as m1pool,
            tc.tile_pool(name="small", bufs=1) as spool,
            tc.tile_pool(name="grp", bufs=2) as gpool,
            tc.tile_pool(name="ps1", bufs=4, space="PSUM") as ps1,
            tc.tile_pool(name="pst", bufs=2, space="PSUM") as pstp,
            tc.tile_pool(name="pso", bufs=1, space="PSUM") as pso,
            tc.tile_pool(name="ps3", bufs=1, space="PSUM") as ps3,
        ):
            # ---- packed small tensors: one DMA on the gpsimd stream ----
            pk = cpool.tile([128, PACKB], U8)
            nc.gpsimd.dma_start(pk[:], PACK[:])
            wsb = pk[:, OFF_W : OFF_W + 528].bitcast(F16)
            wcsb = pk[:, OFF_WC : OFF_WC + 256].bitcast(F32)
            s4sb = pk[:, OFF_S4 : OFF_S4 + 64].bitcast(BF16)
            ptsi = pk[:, OFF_PT : OFF_PT + 512].bitcast(I16)

            id128 = cpool.tile([128, 128], F32)
            masks.make_identity(nc, id128[:])

            const3 = cpool.tile([3, NCELL], F16)

            xres = cpool.tile([128, 2048], F16)
            if mode == "compute":
                nc.sync.dma_start(xres[:], X[0:128, 0:2048])

            def _x_dma(jp, k, eng):
                xt = xpool.tile([128, 2048], F16, tag="xt")
                eng.dma_start(
                    xt[:],
                    X[k * 128 : (k + 1) * 128, jp * 2048 : (jp + 1) * 2048],
                )
                return xt

            def body():
                if mode == "dma":
                    for jp in range(2):
                        for k in range(NCH):
                            _x_dma(jp, k, nc.sync)
                    return

                # X tile DMAs, all issued up front, split across the two
                # sequencer streams. gpsimd first does the scatter chain,
                # then feeds the second half's early k-chunks.
                xts = {}
                if mode != "compute":
                    for k in range(NCH):
                        xts[(0, k)] = _x_dma(0, k, nc.sync)
                    for k in range(5, NCH):
                        xts[(1, k)] = _x_dma(1, k, nc.sync)

                # ---- occupancy: scatter host-deduped indices ----
                ones128 = spool.tile([128, 64], BF16)
                nc.gpsimd.memset(ones128[:], 1.0)
                occ = spool.tile([128, NCELL], BF16)
                for q in range(4):
                    nc.gpsimd.local_scatter(
                        out_ap=occ[:, q * 1024 : (q + 1) * 1024],
                        data_ap=ones128[:],
                        idxs_ap=ptsi[:, q * 64 : (q + 1) * 64],
                        channels=128,
                        num_elems=1024,
                        num_idxs=64,
                    )
                nc.gpsimd.dma_start(const3[:], C3[:])
                if mode != "compute":
                    for k in range(5):
                        xts[(1, k)] = _x_dma(1, k, nc.gpsimd)

                # occ [4*inst+grp, cell] -> occt [cell, inst] (4->1 merge via
                # S4 then clamp to 1).
                occt = cpool.tile([128, NCK * NI], BF16)

                def occ_quarter(q):
                    pso_q = pso.tile([128, 8 * NI], F32, tag="pso")
                    for s in range(8):
                        jj = 8 * q + s
                        nc.tensor.matmul(
                            pso_q[:, s * NI : (s + 1) * NI],
                            occ[:, jj * 128 : (jj + 1) * 128],
                            s4sb[:],
                        )
                    # clamp merged counts to 1 while copying PSUM->SBUF
                    nc.vector.tensor_scalar(
                        occt[:, q * 8 * NI : (q + 1) * 8 * NI],
                        pso_q[:],
                        1.0,
                        None,
                        op0=mybir.AluOpType.min,
                    )

                # at_all: per chunk jj cols [40jj,40jj+40):
                #   0:19 exp(score+aml), 19:38 *vl, 38:40 (wcorr, 1)
                at_all = cpool.tile([128, NCK * 40], BF16)
                atv = at_all[:].rearrange("p (c k) -> p c k", k=40)
                nc.vector.tensor_copy(atv[:, :, 38:40], wcsb[:])

                m1 = m1pool.tile([NOP, NCELL], F32)
                psum3 = ps3.tile([NI, 40], F32)
                for jp in range(2):
                    psums = []
                    for h in range(4):
                        psum_h = ps1.tile([NOP, 512], F32, tag="ps1")
                        psums.append(psum_h)
                    for k in range(NCH):
                        xt = xts[(jp, k)] if mode != "compute" else xres
                        for h in range(4):
                            nc.tensor.matmul(
                                psums[h][:],
                                wsb[:, k * NOP : (k + 1) * NOP],
                                xt[:, h * 512 : (h + 1) * 512],
                                start=(k == 0),
                                stop=False,
                            )
                    for h in range(4):
                        j = 4 * jp + h
                        nc.tensor.matmul(
                            psums[h][:],
                            wsb[0:3, 10 * NOP : 11 * NOP],
                            const3[:, j * 512 : (j + 1) * 512],
                            start=False,
                            stop=True,
                        )
                    occ_quarter(2 * jp)
                    occ_quarter(2 * jp + 1)
                    for h in range(4):
                        j = 4 * jp + h
                        nc.vector.tensor_copy(
                            m1[:, j * 512 : (j + 1) * 512], psums[h][:]
                        )
                        # transpose the 4 chunks of this group: [24,128]->[128,24]
                        pst = pstp.tile([128, 4 * NOP], F32, tag="pst")
                        for s in range(4):
                            jj = 4 * j + s
                            nc.tensor.matmul(
                                pst[:, s * NOP : (s + 1) * NOP],
                                m1[:, jj * 128 : (jj + 1) * 128],
                                id128[0:NOP, 0:NOP],
                                is_transpose=True,
                            )
                        # gather vl / alogit columns, batch aml for 4 chunks
                        pv = pst[:].rearrange("p (s k) -> p s k", k=NOP)
                        vls = gpool.tile([128, 4], F32, tag="vls")
                        av = gpool.tile([128, 4], F32, tag="av")
                        nc.vector.tensor_copy(vls[:], pv[:, :, 19])
                        nc.vector.tensor_copy(av[:], pv[:, :, 20])
                        # aml = ln(sigmoid(z)+eps) = ln(1+eps+eps*e^-z)-ln(1+e^-z)
                        sg = gpool.tile([128, 4], F32, tag="sg")
                        sgw = gpool.tile([128, 4], F32, tag="sgw")
                        nc.scalar.activation(
                            sg[:], av[:],
                            mybir.ActivationFunctionType.Exp, scale=-1.0,
                        )
                        nc.vector.tensor_scalar(
                            sg[:], sg[:], 1.0, None, op0=mybir.AluOpType.add
                        )
                        nc.vector.tensor_scalar(
                            sgw[:], sg[:], EPS, 1.0,
                            op0=mybir.AluOpType.mult, op1=mybir.AluOpType.add,
                        )
                        nc.scalar.activation(
                            sg[:], sg[:], mybir.ActivationFunctionType.Ln
                        )
                        nc.scalar.activation(
                            sgw[:], sgw[:], mybir.ActivationFunctionType.Ln
                        )
                        aml = gpool.tile([128, 4], F32, tag="aml")
                        nc.vector.tensor_tensor(
                            aml[:], sgw[:], sg[:], op=mybir.AluOpType.subtract
                        )
                        for s in range(4):
                            jj = 4 * j + s
                            nc.scalar.activation(
                                atv[:, jj, 0:19],
                                pst[:, s * NOP : s * NOP + 19],
                                mybir.ActivationFunctionType.Exp,
                                bias=aml[:, s : s + 1],
                            )
                            nc.vector.tensor_scalar(
                                atv[:, jj, 19:38],
                                atv[:, jj, 0:19],
                                vls[:, s : s + 1],
                                None,
                                op0=mybir.AluOpType.mult,
                            )
                        for s in range(4):
                            jj = 4 * j + s
                            nc.tensor.matmul(
                                psum3[:],
                                occt[:, jj * NI : (jj + 1) * NI],
                                atv[:, jj, :],
                                start=(jj == 0),
                                stop=(jj == NCK - 1),
                            )

                # ---- finalize: sigmoid(num/den + corr/n) ----
                rsb = spool.tile([NI, 40], F32)
                nc.scalar.copy(rsb[:], psum3[:])
                if dbg:
                    nc.sync.dma_start(M1D[:], m1[:])
                    nc.sync.dma_start(OCTD[:], occt[:])
                    nc.sync.dma_start(PS3D[:], rsb[:])
                t1 = spool.tile([NI, NCAPS], F32)
                t2 = spool.tile([NI, 1], F32)
                rc1 = spool.tile([NI, NCAPS], F32)
                rc2 = spool.tile([NI, 1], F32)
                nc.vector.reciprocal(rc1[:], rsb[:, 0:NCAPS])
                nc.vector.tensor_tensor(
                    t1[:], rsb[:, NCAPS : 2 * NCAPS], rc1[:],
                    op=mybir.AluOpType.mult,
                )
                nc.vector.reciprocal(rc2[:], rsb[:, 39:40])
                nc.vector.tensor_tensor(
                    t2[:], rsb[:, 38:39], rc2[:], op=mybir.AluOpType.mult
                )
                nc.vector.tensor_scalar(
                    t1[:], t1[:], t2[:], None, op0=mybir.AluOpType.add
                )
                # sigmoid(L) = exp(-ln(1+exp(-L))) with only exp/ln
                osb = spool.tile([NI, NCAPS], F32)
                nc.scalar.activation(
                    osb[:], t1[:], mybir.ActivationFunctionType.Exp, scale=-1.0
                )
                nc.vector.tensor_scalar(
                    osb[:], osb[:], 1.0, None, op0=mybir.AluOpType.add
                )
                nc.scalar.activation(
                    osb[:], osb[:], mybir.ActivationFunctionType.Ln
                )
                nc.scalar.activation(
                    osb[:], osb[:], mybir.ActivationFunctionType.Exp, scale=-1.0
                )
                nc.sync.dma_start(OUT[:], osb[:])

            if loop_n == 1:
                body()
            else:
                with tc.For_i(0, loop_n, 1):
                    body()

    nc.compile()
    _CACHE[key] = nc
    return nc


def _fold_weights(Wp, bp, Wa, ba, Q, Wk, bk, Wv, bv, Wl, bl):
    f = lambda t: np.asarray(t, np.float64)
    Wp, bp, Wa, ba, Q, Wk, bk, Wv, bv, Wl, bl = map(
        f, (Wp, bp, Wa, ba, Q, Wk, bk, Wv, bv, Wl, bl)
    )
    wl = Wl[:, 0]
    QT8 = Q.T / 8.0                       # [64,19]
    WK = Wp.T @ Wk[:256]                  # [1280,64]
    wvl_cap = Wv[:256] @ wl               # [256]
    a, b = Wv[256] @ wl, Wv[257] @ wl

    W_all = np.zeros((CIN + 3, NOP), np.float64)
    W_all[:CIN, 0:19] = WK @ QT8
    W_all[:CIN, 19] = Wp.T @ wvl_cap
    W_all[:CIN, 20] = Wa[0]
    W_all[CIN + 0, 0:19] = (Wk[256] / 64.0) @ QT8
    W_all[CIN + 1, 0:19] = (Wk[257] / 64.0) @ QT8
    W_all[CIN + 2, 0:19] = (bp @ Wk[:256] + bk) @ QT8
    W_all[CIN + 0, 19] = a / 64.0
    W_all[CIN + 1, 19] = b / 64.0
    W_all[CIN + 2, 19] = bp @ wvl_cap + bv @ wl
    W_all[CIN + 2, 20] = ba[0]

    c = np.arange(NCELL)
    y64 = (c // 64) / 64.0
    x64 = (c % 64) / 64.0
    wcorr = -(a * y64 + b * x64 - bl[0])
    WC2 = np.empty((128, 2 * NCK), np.float64)
    WC2[:, 0::2] = wcorr.reshape(NCK, 128).T
    WC2[:, 1::2] = 1.0

    return W_all.astype(np.float16), WC2.astype(np.float32)


def _make_in_maps(
    feature_output, Wp, bp, Wa, ba, Q, Wk, bk, Wv, bv, Wl, bl, point_lists
):
    import ml_dtypes

    W_all, WC2 = _fold_weights(Wp, bp, Wa, ba, Q, Wk, bk, Wv, bv, Wl, bl)

    c = np.arange(NCELL)
    C3v = np.stack([c // 64, c % 64, np.ones(NCELL)]).astype(np.float16)

    S4 = np.zeros((128, NI), np.float32)
    S4[np.arange(128), np.arange(128) // 4] = 1.0
    S4 = S4.astype(ml_dtypes.bfloat16)

    # wsb layout [128, 264] fp16: chunk k<10 at cols 24k from W rows 128k+p;
    # tail chunk at cols 240:264 rows 1280:1283 on partitions 0:3.
    wsb = np.zeros((128, 11 * NOP), np.float16)
    for k in range(NCH):
        wsb[:, k * NOP : (k + 1) * NOP] = W_all[k * 128 : (k + 1) * 128]
    wsb[0:3, 10 * NOP : 11 * NOP] = W_all[CIN : CIN + 3]

    fo = np.asarray(feature_output, np.float32).astype(np.float16)

    # Host-deduped scatter indices (see kernel docstring).
    pl = np.asarray(point_lists).astype(np.int64)  # [B, NI, 2, 256]
    keys = (pl[:, :, 0] // 16) * 64 + (pl[:, :, 1] // 16)  # [B, NI, 256]
    ptsi = np.full((B, 128, 256), -1, np.int16)
    for i in range(B):
        for n in range(NI):
            u = np.unique(keys[i, n])
            for g in range(4):
                seg = u[64 * g : 64 * (g + 1)]
                if seg.size == 0:
                    continue
                q = seg // 1024
                ptsi[i, 4 * n + g, q * 64 + np.arange(seg.size) % 64] = (
                    seg - 1024 * q
                )

    def pack_one(i):
        blob = np.zeros((128, PACKB), np.uint8)
        blob[:, OFF_W : OFF_W + 528] = wsb.view(np.uint8).reshape(128, 528)
        blob[:, OFF_WC : OFF_WC + 256] = (
            np.ascontiguousarray(WC2).view(np.uint8).reshape(128, 256)
        )
        blob[:, OFF_S4 : OFF_S4 + 64] = (
            np.ascontiguousarray(S4).view(np.uint8).reshape(128, 64)
        )
        blob[:, OFF_PT : OFF_PT + 512] = (
            np.ascontiguousarray(ptsi[i]).view(np.uint8).reshape(128, 512)
        )
        return blob

    return [
        {
            "X": np.ascontiguousarray(fo[i].reshape(CIN, NCELL)),
            "PACK": pack_one(i),
            "C3": C3v,
        }
        for i in range(B)
    ]


def kernel(
    feature_output, Wp, bp, Wa, ba, Q, Wk, bk, Wv, bv, Wl, bl, point_lists
):
    nc = _build_nc()
    in_maps = _make_in_maps(
        feature_output, Wp, bp, Wa, ba, Q, Wk, bk, Wv, bv, Wl, bl, point_lists
    )
    res = run_bass_kernel_spmd(nc, in_maps, core_ids=list(range(B)))
    return np.stack([res.results[i]["OUT"] for i in range(B)]).astype(np.float32)
